# revision 17
# baseline (speedup 1.0000x reference)
"""Trainium2 Bass kernel for nn_DecoderActor (GCN + GRU encoder/decoder + softmax).

Math (reference):
  net_state = relu(A @ net_x_b @ gcn_w + gcn_b)            per batch b; A shared
  hidden    = GRU_enc over [sfc_state(16); emb(src); emb(dst)]  (18 steps)
  h_t       = GRU_dec(h_{t-1}, node_embed[t])              (16 steps, x batch-indep)
  logits[b,t,n] = ns_proj[b,n] + hdot[b,t] + mlp_b
      where ns_proj[b,n] = net_state[b,n,:] @ mlp_w, hdot[b,t] = h_t[b] @ mlp_w
  softmax over n == softmax(ns_proj) (t-constant shift cancels)

Sharding: pure data parallel, batch 1024 -> 8 cores x 128.

Device layout choices:
  - GRU state h kept transposed [E=128 partitions, b=128 free] all 34 steps.
  - Gate pre-activations accumulated in one PSUM bank [128, 512]:
      cols 0:128 r | 128:256 z | 256:384 i_n | 384:512 h_n
  - GCN: P1 per batch-pair: psum = net_x_pair.T @ A.T (stationary = natural
    net_x!), P2: gcn_w.T @ V.T in bf16, relu fused into psum->sbuf pass,
    ns_proj via [E,1] matmuls, drained by tiny PSUM->SBUF DMAs.
  - Decoder x-side is batch-independent -> folded into per-partition ACT biases.
"""

import os
import sys

sys.path.insert(0, "/opt/trn_rl_repo")

import numpy as np
import ml_dtypes

import concourse.bass as bass
import concourse.bacc as bacc
import concourse.tile as tile
from concourse import mybir
from concourse.bass_utils import run_bass_kernel_spmd

F32 = mybir.dt.float32
BF16 = mybir.dt.bfloat16
AF = mybir.ActivationFunctionType
ALU = mybir.AluOpType

B, N, T, E, F_NET, NUM_NODES = 1024, 128, 16, 128, 64, 128
NCORES = 8
BS = B // NCORES  # 128 batch per core
NPAIR = BS // 2   # 64 pairs
NTILE = NPAIR // 4  # 16 pair-tiles

_CACHE = {}


def _build_program():
    nc = bacc.Bacc(
        "TRN2",
        target_bir_lowering=False,
        debug=False,
        num_swdge_queues=4,
    )

    # ---------------- DRAM I/O ----------------
    dram = {}

    def din(name, shape, dt=F32):
        dram[name] = nc.dram_tensor(name, list(shape), dt, kind="ExternalInput").ap()
        return dram[name]

    netx_d = din("netx", [BS, N, F_NET])          # [b, s, f]
    sfc_d = din("sfc", [BS, T, E])                # [b, t, e]
    at_d = din("at", [N, N])                      # A.T  [s, d]
    gcnw_d = din("gcnw", [2 * F_NET, E], BF16)  # gcn_w stacked twice
    gcnb_d = din("gcnb", [E, 1])
    mlpw_d = din("mlpw", [E, 1], BF16)
    onehot_d = din("onehot", [E, 16 * T])         # col t*16+t = mlp_w
    ohproj_d = din("ohproj", [E, 32 * 32], BF16)  # stat g: col g = mlp_w
    wihTe_d = din("wihTe", [E, 3 * E])            # enc_wih.T
    whhTe_d = din("whhTe", [E, 3 * E])            # enc_whh.T
    whhTd_d = din("whhTd", [E, 3 * E])            # dec_whh.T
    embs_d = din("embs", [E, BS])                 # emb(src).T core slice
    embd_d = din("embd", [E, BS])                 # emb(dst).T core slice
    brE_d = din("brE", [E, 1])
    bzE_d = din("bzE", [E, 1])
    binE_d = din("binE", [E, 1])
    bhnE_d = din("bhnE", [E, 1])
    brD_d = din("brD", [E, T])                    # per-step dec r bias
    bzD_d = din("bzD", [E, T])
    btD_d = din("btD", [E, T])                    # per-step dec tanh bias
    mscD_d = din("mscD", [E, 1])                  # dec_bhh_n
    mlpb16_d = din("mlpb16", [T, 1])              # mlp_b replicated
    eye_d = din("eye", [128, 128])

    logits_o = nc.dram_tensor("logits", [BS, T * N], F32, kind="ExternalOutput").ap()
    soft_o = nc.dram_tensor("soft", [BS, T * N], F32, kind="ExternalOutput").ap()

    from contextlib import ExitStack

    with tile.TileContext(nc) as tc, ExitStack() as ctx:
        # ---------------- pools ----------------
        persist = ctx.enter_context(tc.tile_pool(name="persist", bufs=1))
        netx_pool = ctx.enter_context(tc.tile_pool(name="netx", bufs=4))
        sfc_pool = ctx.enter_context(tc.tile_pool(name="sfcload", bufs=3))
        vt_pool = ctx.enter_context(tc.tile_pool(name="vt", bufs=3))
        nst_pool = ctx.enter_context(tc.tile_pool(name="nst", bufs=4))
        gru_pool = ctx.enter_context(tc.tile_pool(name="gru", bufs=2))
        # psum pools: p1(2) + p2(2) + gps(2) + proj(1) + hd(1) = 8 banks
        p1_ps = ctx.enter_context(tc.tile_pool(name="p1ps", bufs=2, space="PSUM"))
        p2_ps = ctx.enter_context(tc.tile_pool(name="p2ps", bufs=2, space="PSUM"))
        g_ps = ctx.enter_context(tc.tile_pool(name="gps", bufs=2, space="PSUM"))
        pr_ps = ctx.enter_context(tc.tile_pool(name="prps", bufs=1, space="PSUM"))
        hd_ps = ctx.enter_context(tc.tile_pool(name="hdps", bufs=1, space="PSUM"))

        def pt(name, shape, dt=F32):
            return persist.tile(list(shape), dt, tag=name, name=name)

        # ---------------- const loads ----------------
        at_t = pt("at", [N, N])
        gcnw_t = pt("gcnw", [2 * F_NET, E], BF16)
        gcnb_t = pt("gcnb", [E, 1])
        mlpw_t = pt("mlpw", [E, 1], BF16)
        onehot_t = pt("onehot", [E, 16 * T])
        ohproj_t = pt("ohproj", [E, 32 * 32], BF16)
        prsb_t = pt("prsb", [32, 512])
        wihTe_t = pt("wihTe", [E, 3 * E])
        whhTe_t = pt("whhTe", [E, 3 * E])
        whhTd_t = pt("whhTd", [E, 3 * E])
        brE_t = pt("brE", [E, 1])
        bzE_t = pt("bzE", [E, 1])
        binE_t = pt("binE", [E, 1])
        bhnE_t = pt("bhnE", [E, 1])
        brD_t = pt("brD", [E, T])
        bzD_t = pt("bzD", [E, T])
        btD_t = pt("btD", [E, T])
        mscD_t = pt("mscD", [E, 1])
        mlpb16_t = pt("mlpb16", [T, 1])
        eye_t = pt("eye", [128, 128])
        nsproj_t = pt("nsproj", [BS, N])
        hdot_t = pt("hdot", [BS, T])
        hd1_t = pt("hd1", [T, BS])
        logits_all = pt("logits_all", [BS, T * N])
        soft_all = pt("soft_all", [BS, T * N])

        for tl, d in [
            (at_t, at_d), (gcnw_t, gcnw_d), (gcnb_t, gcnb_d), (mlpw_t, mlpw_d),
            (onehot_t, onehot_d), (wihTe_t, wihTe_d), (whhTe_t, whhTe_d),
            (whhTd_t, whhTd_d), (brE_t, brE_d), (bzE_t, bzE_d), (binE_t, binE_d),
            (bhnE_t, bhnE_d), (brD_t, brD_d), (bzD_t, bzD_d), (btD_t, btD_d),
            (mscD_t, mscD_d), (mlpb16_t, mlpb16_d), (eye_t, eye_d),
            (ohproj_t, ohproj_d),
        ]:
            nc.sync.dma_start(out=tl[:], in_=d[:])

        xt = [pt(f"xt{t}", [E, BS]) for t in range(T)]
        embs_t = pt("embs", [E, BS])
        embd_t = pt("embd", [E, BS])
        nc.sync.dma_start(out=embs_t[:], in_=embs_d[:])
        nc.sync.dma_start(out=embd_t[:], in_=embd_d[:])

        # ---------------- sfc loads + PE transposes ----------------
        for t in range(T):
            s_tile = sfc_pool.tile([BS, E], F32, tag="sfc")
            nc.sync.dma_start(out=s_tile[:], in_=sfc_d[:, t, :])
            tp = g_ps.tile([E, BS], F32, tag="gps")
            nc.tensor.transpose(tp[:], s_tile[:], eye_t[:])
            if t % 2 == 0:
                nc.vector.tensor_copy(xt[t][:], tp[:])
            else:
                nc.scalar.copy(xt[t][:], tp[:])

        # ---------------- GCN ----------------
        # pair p = (p, p+64); tile j holds pairs 4j..4j+3
        prp = pr_ps.tile([128, 512], F32, tag="proj", name="prp")
        for j in range(NTILE):
            p1 = p1_ps.tile([128, 512], F32, tag="p1")
            for k in range(4):
                p = 4 * j + k
                stat = netx_pool.tile([N, 128], F32, tag="netx")
                nc.sync.dma_start(out=stat[:, 0:F_NET], in_=netx_d[p])
                nc.sync.dma_start(out=stat[:, F_NET:128], in_=netx_d[p + NPAIR])
                # psum[(bh,f), d] = sum_s net_x[b,s,f] * AT[s,d]  = V_b.T
                nc.tensor.matmul(
                    p1[:, k * 128:(k + 1) * 128], stat[:], at_t[:],
                    start=True, stop=True,
                )
            vt = vt_pool.tile([128, 512], BF16, tag="vt")
            if j % 2 == 0:
                nc.vector.tensor_copy(vt[:], p1[:])
            else:
                nc.scalar.copy(vt[:], p1[:])
            # P2: rows 0:64 -> bs {4j..4j+3}, rows 64:128 -> {4j+64..}
            for half in range(2):
                p2 = p2_ps.tile([E, 512], F32, tag="p2")
                nc.tensor.matmul(
                    p2[:], gcnw_t[64 * half:64 * (half + 1), :],
                    vt[64 * half:64 * (half + 1), :],
                    start=True, stop=True,
                )
                nst = nst_pool.tile([E, 512], BF16, tag="nst")
                if (2 * j + half) % 2 == 0:
                    nc.scalar.activation(nst[:], p2[:], AF.Relu, bias=gcnb_t[:, 0:1])
                else:
                    nc.vector.tensor_scalar(
                        nst[:], p2[:], gcnb_t[:, 0:1], 0.0, ALU.add, ALU.max
                    )
                # ns_proj: one-hot col g -> proj values land at psum row g;
                # all 32 matmuls accumulate into one bank. b = 4*g + q.
                g = j + 16 * half
                nc.tensor.matmul(
                    prp[0:32, :], ohproj_t[:, g * 32:(g + 1) * 32], nst[:],
                    start=(g == 0), stop=(g == 31),
                    skip_group_check=True,
                )

        # drain ns_proj: psum rows g -> sbuf -> rearrange to [b, d] (b = 4g+q)
        nc.vector.tensor_copy(prsb_t[:], prp[0:32, :])
        nc.sync.dma_start(out=nsproj_t[:], in_=prsb_t[:])

        # ---------------- GRU helpers ----------------
        def gru_step(h_prev, x_t, wT_h, wT_x, b_r, b_z, tanh_bias, m_scalar,
                     enc: bool):
            """One GRU step in [E, b] layout. Returns new h tile.

            enc: x-side via matmuls (wT_x, x_t) accumulated into psum.
            dec: x-side folded into biases (wT_x, x_t unused).
            """
            P = g_ps.tile([128, 512], F32, tag="gps")
            # r
            nc.tensor.matmul(P[:, 0:128], wT_h[:, 0:128], h_prev[:],
                             start=True, stop=not enc)
            if enc:
                nc.tensor.matmul(P[:, 0:128], wT_x[:, 0:128], x_t[:],
                                 start=False, stop=True)
            # z
            nc.tensor.matmul(P[:, 128:256], wT_h[:, 128:256], h_prev[:],
                             start=True, stop=not enc)
            if enc:
                nc.tensor.matmul(P[:, 128:256], wT_x[:, 128:256], x_t[:],
                                 start=False, stop=True)
            # i_n (enc only)
            if enc:
                nc.tensor.matmul(P[:, 256:384], wT_x[:, 256:384], x_t[:],
                                 start=True, stop=True)
            # h_n
            nc.tensor.matmul(P[:, 384:512], wT_h[:, 256:384], h_prev[:],
                             start=True, stop=True)

            rz = gru_pool.tile([128, 256], F32, tag="rz")
            nc.scalar.activation(rz[:, 0:128], P[:, 0:128], AF.Sigmoid, bias=b_r)
            nc.scalar.activation(rz[:, 128:256], P[:, 128:256], AF.Sigmoid, bias=b_z)
            # m = (h_n + bhh_n) * r
            m_t = gru_pool.tile([128, BS], F32, tag="m")
            nc.vector.scalar_tensor_tensor(
                m_t[:], P[:, 384:512], m_scalar, rz[:, 0:128], ALU.add, ALU.mult
            )
            if enc:
                s_t = gru_pool.tile([128, BS], F32, tag="s")
                nc.vector.tensor_tensor(s_t[:], m_t[:], P[:, 256:384], ALU.add)
            else:
                s_t = m_t
            n_t = gru_pool.tile([128, BS], F32, tag="n")
            nc.scalar.activation(n_t[:], s_t[:], AF.Tanh, bias=tanh_bias)
            # h' = z*h - (z-1)*n  = (1-z)*n + z*h
            zh = gru_pool.tile([128, BS], F32, tag="zh")
            nc.vector.tensor_tensor(zh[:], rz[:, 128:256], h_prev[:], ALU.mult)
            zn = gru_pool.tile([128, BS], F32, tag="zn")
            nc.vector.scalar_tensor_tensor(
                zn[:], rz[:, 128:256], 1.0, n_t[:], ALU.subtract, ALU.mult
            )
            h_new = gru_pool.tile([128, BS], F32, tag="h")
            nc.vector.tensor_tensor(h_new[:], zh[:], zn[:], ALU.subtract)
            return h_new

        # ---------------- encoder ----------------
        h = gru_pool.tile([128, BS], F32, tag="h")
        nc.vector.memset(h[:], 0.0)
        enc_xs = xt + [embs_t, embd_t]
        for t in range(T + 2):
            h = gru_step(h, enc_xs[t], whhTe_t, wihTe_t,
                         brE_t[:, 0:1], bzE_t[:, 0:1], binE_t[:, 0:1],
                         bhnE_t[:, 0:1], enc=True)

        # ---------------- decoder ----------------
        hd = hd_ps.tile([T, BS], F32, tag="hd")
        for t in range(T):
            h = gru_step(h, None, whhTd_t, None,
                         brD_t[:, t:t + 1], bzD_t[:, t:t + 1], btD_t[:, t:t + 1],
                         mscD_t[:, 0:1], enc=False)
            nc.tensor.matmul(hd[:], onehot_t[:, t * 16:(t + 1) * 16], h[:],
                             start=(t == 0), stop=(t == T - 1))

        # ---------------- hdot drain: [t,b] -> [b,t] ----------------
        nc.vector.tensor_scalar(hd1_t[:], hd[:], mlpb16_t[:, 0:1], None, ALU.add)
        hdp = g_ps.tile([BS, T], F32, tag="gps")
        nc.tensor.transpose(hdp[:], hd1_t[:], eye_t[0:T, 0:T])
        nc.vector.tensor_copy(hdot_t[:], hdp[:])

        # ---------------- softmax (t-independent) ----------------
        mx = pt("mx", [BS, 1])
        nc.vector.reduce_max(mx[:], nsproj_t[:], mybir.AxisListType.X)
        negmx = pt("negmx", [BS, 1])
        nc.vector.tensor_scalar(negmx[:], mx[:], -1.0, None, ALU.mult)
        ex = pt("ex", [BS, N])
        ssum = pt("ssum", [BS, 1])
        nc.scalar.activation(ex[:], nsproj_t[:], AF.Exp, bias=negmx[:, 0:1],
                             accum_out=ssum[:, 0:1])
        rsum = pt("rsum", [BS, 1])
        nc.vector.reciprocal(rsum[:], ssum[:])

        for t in range(T):
            nc.vector.tensor_scalar(
                soft_all[:, t * N:(t + 1) * N], ex[:], rsum[:, 0:1], None, ALU.mult
            )
            nc.vector.tensor_scalar(
                logits_all[:, t * N:(t + 1) * N], nsproj_t[:], hdot_t[:, t:t + 1],
                None, ALU.add
            )

        nc.sync.dma_start(out=logits_o[:], in_=logits_all[:])
        nc.sync.dma_start(out=soft_o[:], in_=soft_all[:])

    nc.finalize()  # Bacc.compile(): wait-splitting, reg alloc, nop fusion
    return nc


def _prep_inputs(inputs):
    """Host-side preprocessing -> per-core input maps."""
    f32 = np.float32
    net_x = np.ascontiguousarray(np.asarray(inputs["net_x"], dtype=f32))
    sfc_state = np.ascontiguousarray(np.asarray(inputs["sfc_state"], dtype=f32))
    edge_index = np.asarray(inputs["edge_index"]).astype(np.int64)
    source_dest = np.asarray(inputs["source_dest"]).astype(np.int64)
    node_embed = np.asarray(inputs["node_embed"], dtype=f32)
    gcn_w = np.asarray(inputs["gcn_w"], dtype=f32)
    gcn_b = np.asarray(inputs["gcn_b"], dtype=f32)
    enc_wih = np.asarray(inputs["enc_wih"], dtype=f32)
    enc_whh = np.asarray(inputs["enc_whh"], dtype=f32)
    enc_bih = np.asarray(inputs["enc_bih"], dtype=f32)
    enc_bhh = np.asarray(inputs["enc_bhh"], dtype=f32)
    dec_wih = np.asarray(inputs["dec_wih"], dtype=f32)
    dec_whh = np.asarray(inputs["dec_whh"], dtype=f32)
    dec_bih = np.asarray(inputs["dec_bih"], dtype=f32)
    dec_bhh = np.asarray(inputs["dec_bhh"], dtype=f32)
    mlp_w = np.asarray(inputs["mlp_w"], dtype=f32)
    mlp_b = np.asarray(inputs["mlp_b"], dtype=f32)

    # normalized adjacency with self-loops, transposed: AT[s, d]
    src = np.concatenate([edge_index[0], np.arange(N, dtype=np.int64)])
    dst = np.concatenate([edge_index[1], np.arange(N, dtype=np.int64)])
    deg = np.zeros(N, dtype=f32)
    np.add.at(deg, dst, f32(1.0))
    with np.errstate(divide="ignore"):
        dinv = (1.0 / np.sqrt(deg)).astype(f32)
    norm = (dinv[src] * dinv[dst]).astype(f32)
    AT = np.zeros((N, N), dtype=f32)
    np.add.at(AT, (src, dst), norm)

    embs_full = np.ascontiguousarray(node_embed[source_dest[:, 0]].T)  # [E, B]
    embd_full = np.ascontiguousarray(node_embed[source_dest[:, 1]].T)

    gi_dec = node_embed[:T] @ dec_wih.T + dec_bih  # [T, 384]
    brD = np.ascontiguousarray(gi_dec[:, 0:E].T + dec_bhh[0:E, None])
    bzD = np.ascontiguousarray(gi_dec[:, E:2 * E].T + dec_bhh[E:2 * E, None])
    btD = np.ascontiguousarray(gi_dec[:, 2 * E:3 * E].T)
    mscD = np.ascontiguousarray(dec_bhh[2 * E:3 * E, None])

    onehot = np.zeros((E, 16 * T), dtype=f32)
    for t in range(T):
        onehot[:, t * 16 + t] = mlp_w

    mlp_w_bf = mlp_w.astype(ml_dtypes.bfloat16)
    ohproj = np.zeros((E, 32 * 32), dtype=ml_dtypes.bfloat16)
    for g in range(32):
        ohproj[:, g * 32 + g] = mlp_w_bf

    shared = {
        "at": AT,
        "gcnw": np.vstack([gcn_w, gcn_w]).astype(ml_dtypes.bfloat16),
        "gcnb": np.ascontiguousarray(gcn_b[:, None]),
        "mlpw": mlp_w.astype(ml_dtypes.bfloat16)[:, None].copy(),
        "onehot": onehot,
        "ohproj": ohproj,
        "wihTe": np.ascontiguousarray(enc_wih.T),
        "whhTe": np.ascontiguousarray(enc_whh.T),
        "whhTd": np.ascontiguousarray(dec_whh.T),
        "brE": np.ascontiguousarray((enc_bih[0:E] + enc_bhh[0:E])[:, None]),
        "bzE": np.ascontiguousarray((enc_bih[E:2 * E] + enc_bhh[E:2 * E])[:, None]),
        "binE": np.ascontiguousarray(enc_bih[2 * E:3 * E, None]),
        "bhnE": np.ascontiguousarray(enc_bhh[2 * E:3 * E, None]),
        "brD": brD,
        "bzD": bzD,
        "btD": btD,
        "mscD": mscD,
        "mlpb16": np.full((T, 1), float(mlp_b), dtype=f32),
        "eye": np.eye(128, dtype=f32),
    }
    in_maps = []
    for c in range(NCORES):
        b0 = c * BS
        m = dict(shared)
        m["netx"] = np.ascontiguousarray(net_x[b0:b0 + BS])
        m["sfc"] = np.ascontiguousarray(sfc_state[b0:b0 + BS])
        m["embs"] = np.ascontiguousarray(embs_full[:, b0:b0 + BS])
        m["embd"] = np.ascontiguousarray(embd_full[:, b0:b0 + BS])
        in_maps.append(m)
    return in_maps


def _get_program():
    if "nc" not in _CACHE:
        _CACHE["nc"] = _build_program()
    return _CACHE["nc"]


def run(inputs, trace=False, tmpdir=None):
    nc = _get_program()
    in_maps = _prep_inputs(inputs)
    res = run_bass_kernel_spmd(
        nc, in_maps, list(range(NCORES)), trace=trace, tmpdir=tmpdir
    )
    logits = np.concatenate(
        [res.results[c]["logits"].reshape(BS, T, N) for c in range(NCORES)], axis=0
    )
    soft = np.concatenate(
        [res.results[c]["soft"].reshape(BS, T, N) for c in range(NCORES)], axis=0
    )
    return (logits, soft), res


def kernel(**inputs):
    (logits, soft), _ = run(inputs, trace=False)
    return logits, soft


# revision 20
# speedup vs baseline: 1.1105x; 1.1105x over previous
"""Trainium2 Bass kernel for nn_DecoderActor (GCN + GRU encoder/decoder + softmax).

Math (reference):
  net_state = relu(A @ net_x_b @ gcn_w + gcn_b)            per batch b; A shared
  hidden    = GRU_enc over [sfc_state(16); emb(src); emb(dst)]  (18 steps)
  h_t       = GRU_dec(h_{t-1}, node_embed[t])              (16 steps, x batch-indep)
  logits[b,t,n] = ns_proj[b,n] + hdot[b,t] + mlp_b
      where ns_proj[b,n] = net_state[b,n,:] @ mlp_w, hdot[b,t] = h_t[b] @ mlp_w
  softmax over n == softmax(ns_proj) (the (b,t)-constant shift cancels exactly)

Sharding: pure data parallel, batch 1024 -> 8 cores x 128.

Device design (v2):
  - All matmuls bf16 (fp32 PSUM accumulation); fp32->bf16 casts ride gpsimd
    cast-DMAs and psum->sbuf copies, costing nothing extra.
  - GRU state h kept transposed [E=128 partitions, b free]; batch split in two
    64-wide chunks that pipeline through PE/ACT/DVE/GPSIMD to hide the serial
    per-step latency (sem hops) of the recurrence.
  - Gate pre-activations per chunk in one PSUM bank [128, 256]:
      cols 0:64 r | 64:128 z | 128:192 h_n | 192:256 i_n(enc only)
    x-side biases enter via a K<=2 "preload" matmul (bias rows x select mask),
    so sigmoid(r|z) is ONE ACT op with no bias.
  - GCN: P1 per batch-pair (stationary = natural-layout net_x pair, moving =
    A.T), P2 = gcn_w.T @ V.T at N=512, relu+bias fused into the psum->sbuf
    cast, ns_proj via one-hot stationaries accumulating all 16K values into a
    single PSUM bank, drained by one copy + one SBUF rearrange DMA.
"""

import numpy as np
import ml_dtypes

import concourse.bass as bass
import concourse.bacc as bacc
import concourse.tile as tile
from concourse import mybir
from concourse.bass_utils import run_bass_kernel_spmd

F32 = mybir.dt.float32
BF16 = mybir.dt.bfloat16
AF = mybir.ActivationFunctionType
ALU = mybir.AluOpType

B, N, T, E, F_NET, NUM_NODES = 1024, 128, 16, 128, 64, 128
NCORES = 8
BS = B // NCORES     # 128 batch per core
NPAIR = BS // 2      # 64 pairs
NTILE = NPAIR // 4   # 16 pair-tiles
CW = 64              # GRU chunk width
NCH = BS // CW       # 2 chunks

_CACHE = {}


def _build_program(enc_preload: bool):
    nc = bacc.Bacc(
        "TRN2",
        target_bir_lowering=False,
        debug=False,
        num_swdge_queues=4,
    )

    # ---------------- DRAM I/O ----------------
    def din(name, shape, dt=F32):
        return nc.dram_tensor(name, list(shape), dt, kind="ExternalInput").ap()

    netx_d = din("netx", [BS, N, F_NET])            # [b, s, f] f32
    sfc_d = din("sfc", [BS, T, E])                  # [b, t, e] f32
    at_d = din("at", [N, N], BF16)                  # A.T  [s, d]
    gcnw_d = din("gcnw", [2 * F_NET, E], BF16)      # gcn_w stacked twice
    gcnb_d = din("gcnb", [E, 1])
    onehot_d = din("onehot", [E, 16 * T], BF16)     # col t*16+t = mlp_w
    ohproj_d = din("ohproj", [E, 32 * 32], BF16)    # stat g: col g*32+g = mlp_w
    wihTe_d = din("wihTe", [E, 3 * E], BF16)        # enc_wih.T
    whhTe_d = din("whhTe", [E, 3 * E], BF16)        # enc_whh.T
    whhTd_d = din("whhTd", [E, 3 * E], BF16)        # dec_whh.T
    embs_d = din("embs", [E, BS])                   # emb(src).T slice, f32
    embd_d = din("embd", [E, BS])
    encpre_d = din("encpre", [2, E], BF16)          # [brE | bzE] rows
    decpre_d = din("decpre", [2, T * E], BF16)      # per-step [brD | bzD]
    selmask_d = din("selmask", [2, 2 * CW], BF16)   # row0: ones 0:CW; row1: CW:2CW
    binE_d = din("binE", [E, 1])
    bhnE_d = din("bhnE", [E, 1])
    btD_d = din("btD", [E, T])                      # dec tanh bias per step
    mscD_d = din("mscD", [E, 1])                    # dec_bhh_n
    mlpb16_d = din("mlpb16", [T, 1])                # mlp_b replicated
    eye_d = din("eye", [128, 128], BF16)

    logits_o = nc.dram_tensor("logits", [BS, T * N], F32, kind="ExternalOutput").ap()
    soft_o = nc.dram_tensor("soft", [BS, T * N], F32, kind="ExternalOutput").ap()

    from contextlib import ExitStack

    with tile.TileContext(nc) as tc, ExitStack() as ctx:
        # ---------------- pools ----------------
        persist = ctx.enter_context(tc.tile_pool(name="persist", bufs=1))
        netx_pool = ctx.enter_context(tc.tile_pool(name="netx", bufs=3))
        sfc_pool = ctx.enter_context(tc.tile_pool(name="sfcload", bufs=3))
        vt_pool = ctx.enter_context(tc.tile_pool(name="vt", bufs=3))
        nst_pool = ctx.enter_context(tc.tile_pool(name="nst", bufs=4))
        gru_pool = ctx.enter_context(tc.tile_pool(name="gru", bufs=3))
        # psum banks: gps(3) + p1(2) + p2(1) + proj(1) + hd(1) = 8
        g_ps = ctx.enter_context(tc.tile_pool(name="gps", bufs=3, space="PSUM"))
        p1_ps = ctx.enter_context(tc.tile_pool(name="p1ps", bufs=2, space="PSUM"))
        p2_ps = ctx.enter_context(tc.tile_pool(name="p2ps", bufs=1, space="PSUM"))
        pr_ps = ctx.enter_context(tc.tile_pool(name="prps", bufs=1, space="PSUM"))
        hd_ps = ctx.enter_context(tc.tile_pool(name="hdps", bufs=1, space="PSUM"))

        def pt(name, shape, dt=F32):
            return persist.tile(list(shape), dt, tag=name, name=name)

        # ---------------- const tiles + loads ----------------
        at_t = pt("at", [N, N], BF16)
        gcnw_t = pt("gcnw", [2 * F_NET, E], BF16)
        gcnb_t = pt("gcnb", [E, 1])
        onehot_t = pt("onehot", [E, 16 * T], BF16)
        ohproj_t = pt("ohproj", [E, 32 * 32], BF16)
        prsb_t = pt("prsb", [32, 512])
        wihTe_t = pt("wihTe", [E, 3 * E], BF16)
        whhTe_t = pt("whhTe", [E, 3 * E], BF16)
        whhTd_t = pt("whhTd", [E, 3 * E], BF16)
        encpre_t = pt("encpre", [2, E], BF16)
        decpre_t = pt("decpre", [2, T * E], BF16)
        selmask_t = pt("selmask", [2, 2 * CW], BF16)
        binE_t = pt("binE", [E, 1])
        bhnE_t = pt("bhnE", [E, 1])
        btD_t = pt("btD", [E, T])
        mscD_t = pt("mscD", [E, 1])
        mlpb16_t = pt("mlpb16", [T, 1])
        eye_t = pt("eye", [128, 128], BF16)
        nsproj_t = pt("nsproj", [BS, N])
        hdot_t = pt("hdot", [BS, T])
        hd1_t = pt("hd1", [T, BS])
        logits_all = pt("logits_all", [BS, T * N])
        soft_all = pt("soft_all", [BS, T * N])

        for tl, d in [
            (at_t, at_d), (gcnw_t, gcnw_d), (gcnb_t, gcnb_d),
            (onehot_t, onehot_d), (ohproj_t, ohproj_d), (wihTe_t, wihTe_d),
            (whhTe_t, whhTe_d), (whhTd_t, whhTd_d), (encpre_t, encpre_d),
            (decpre_t, decpre_d), (selmask_t, selmask_d), (binE_t, binE_d),
            (bhnE_t, bhnE_d), (btD_t, btD_d), (mscD_t, mscD_d),
            (mlpb16_t, mlpb16_d), (eye_t, eye_d),
        ]:
            nc.sync.dma_start(out=tl[:], in_=d[:])

        xt = [pt(f"xt{t}", [E, BS], BF16) for t in range(T)]
        embs_t = pt("embs", [E, BS], BF16)
        embd_t = pt("embd", [E, BS], BF16)
        nc.gpsimd.dma_start(out=embs_t[:], in_=embs_d[:])  # f32 -> bf16 cast DMA
        nc.gpsimd.dma_start(out=embd_t[:], in_=embd_d[:])

        # ---------------- sfc loads (cast) + PE transposes ----------------
        for t in range(T):
            s_tile = sfc_pool.tile([BS, E], BF16, tag="sfc")
            nc.gpsimd.dma_start(out=s_tile[:], in_=sfc_d[:, t, :])
            tp = g_ps.tile([E, BS], BF16, tag="gps")
            nc.tensor.transpose(tp[:], s_tile[:], eye_t[:])
            if t % 2 == 0:
                nc.vector.tensor_copy(xt[t][:], tp[:])
            else:
                nc.scalar.copy(xt[t][:], tp[:])

        # ---------------- GCN ----------------
        # pair p = (p, p+64); tile j holds pairs 4j..4j+3
        prp = pr_ps.tile([128, 512], F32, tag="proj", name="prp")
        for j in range(NTILE):
            stat4 = netx_pool.tile([N, 512], BF16, tag="netx", name=f"nx{j}")
            d4 = stat4[:].rearrange("s (k c) -> s k c", k=4)
            nc.gpsimd.dma_start(
                out=d4[:, :, 0:F_NET],
                in_=netx_d[4 * j:4 * j + 4].rearrange("b s f -> s b f"),
            )
            nc.gpsimd.dma_start(
                out=d4[:, :, F_NET:128],
                in_=netx_d[4 * j + NPAIR:4 * j + NPAIR + 4].rearrange(
                    "b s f -> s b f"
                ),
            )
            p1 = p1_ps.tile([128, 512], F32, tag="p1")
            for k in range(4):
                # psum[(bh,f), d] = sum_s net_x[b,s,f] * AT[s,d] = V_b.T
                nc.tensor.matmul(
                    p1[:, k * 128:(k + 1) * 128],
                    stat4[:, k * 128:(k + 1) * 128], at_t[:],
                    start=(k == 0), stop=(k == 3), skip_group_check=True,
                )
            vt = vt_pool.tile([128, 512], BF16, tag="vt")
            if j % 2 == 0:
                nc.vector.tensor_copy(vt[:], p1[:])
            else:
                nc.scalar.copy(vt[:], p1[:])
            # P2: rows 0:64 -> bs {4j..4j+3}, rows 64:128 -> {4j+64..4j+67}
            for half in range(2):
                p2 = p2_ps.tile([E, 512], F32, tag="p2")
                nc.tensor.matmul(
                    p2[:], gcnw_t[64 * half:64 * (half + 1), :],
                    vt[64 * half:64 * (half + 1), :],
                    start=True, stop=True,
                )
                nst = nst_pool.tile([E, 512], BF16, tag="nst")
                if (2 * j + half) % 3 != 0:
                    nc.scalar.activation(nst[:], p2[:], AF.Relu, bias=gcnb_t[:, 0:1])
                else:
                    nc.vector.tensor_scalar(
                        nst[:], p2[:], gcnb_t[:, 0:1], 0.0, ALU.add, ALU.max
                    )
                # ns_proj via one-hot column g: values land at psum row g.
                # b = 4*g + q  (g = j + 16*half, q = col block // 128)
                g = j + 16 * half
                nc.tensor.matmul(
                    prp[0:32, :], ohproj_t[:, g * 32:(g + 1) * 32], nst[:],
                    start=(g == 0), stop=(g == 31),
                    skip_group_check=True,
                )

        # drain ns_proj: psum rows g -> sbuf -> rearrange to [b, d] (b = 4g+q)
        nc.vector.tensor_copy(prsb_t[:], prp[0:32, :])
        nc.sync.dma_start(out=nsproj_t[:], in_=prsb_t[:])

        # ---------------- GRU (2-chunk pipelined) ----------------
        # psum bank layout per chunk-step [128, 256]:
        #   r 0:CW | z CW:2CW | h_n 2CW:3CW | i_n 3CW:4CW (enc only)
        def gru_step(h_prev, x_t, c, wT_h, wT_x, pre_stat, tanh_bias, m_scalar,
                     enc: bool):
            lo, hi = c * CW, (c + 1) * CW
            P = g_ps.tile([128, 4 * CW], F32, tag="gps")
            use_pre = pre_stat is not None
            if use_pre:
                # bias preload: psum[u, 0:2CW] = br[u]|bz[u] via select mask
                nc.tensor.matmul(P[:, 0:2 * CW], pre_stat, selmask_t[:],
                                 start=True, stop=False)
            # exactly ONE start=True per bank fill: start clears has_written
            # for the whole bank; later mms overwrite virgin regions and
            # accumulate onto already-written ones.
            nc.tensor.matmul(P[:, 0:CW], wT_h[:, 0:128], h_prev[:],
                             start=not use_pre, stop=False,
                             skip_group_check=True)
            if enc:
                nc.tensor.matmul(P[:, 0:CW], wT_x[:, 0:128], x_t[:, lo:hi],
                                 start=False, stop=False, skip_group_check=True)
            # z
            nc.tensor.matmul(P[:, CW:2 * CW], wT_h[:, 128:256], h_prev[:],
                             start=False, stop=False, skip_group_check=True)
            if enc:
                nc.tensor.matmul(P[:, CW:2 * CW], wT_x[:, 128:256], x_t[:, lo:hi],
                                 start=False, stop=False, skip_group_check=True)
            # h_n
            nc.tensor.matmul(P[:, 2 * CW:3 * CW], wT_h[:, 256:384], h_prev[:],
                             start=False, stop=not enc, skip_group_check=True)
            # i_n (enc only)
            if enc:
                nc.tensor.matmul(P[:, 3 * CW:4 * CW], wT_x[:, 256:384],
                                 x_t[:, lo:hi], start=False, stop=True,
                                 skip_group_check=True)

            rz = gru_pool.tile([128, 2 * CW], F32, tag=f"rz{c}")
            nc.scalar.activation(rz[:], P[:, 0:2 * CW], AF.Sigmoid)
            # m = (h_n + bhh_n) * r
            m_t = gru_pool.tile([128, CW], F32, tag=f"m{c}")
            nc.vector.scalar_tensor_tensor(
                m_t[:], P[:, 2 * CW:3 * CW], m_scalar, rz[:, 0:CW],
                ALU.add, ALU.mult
            )
            if enc:
                s_t = gru_pool.tile([128, CW], F32, tag=f"s{c}")
                nc.vector.tensor_tensor(s_t[:], m_t[:], P[:, 3 * CW:4 * CW],
                                        ALU.add)
            else:
                s_t = m_t
            n_t = gru_pool.tile([128, CW], F32, tag=f"n{c}")
            nc.scalar.activation(n_t[:], s_t[:], AF.Tanh, bias=tanh_bias)
            # h' = z*h - (z-1)*n
            zh = gru_pool.tile([128, CW], F32, tag=f"zh{c}")
            nc.gpsimd.tensor_tensor(zh[:], rz[:, CW:2 * CW], h_prev[:], ALU.mult)
            zn = gru_pool.tile([128, CW], F32, tag=f"zn{c}")
            nc.vector.scalar_tensor_tensor(
                zn[:], rz[:, CW:2 * CW], 1.0, n_t[:], ALU.subtract, ALU.mult
            )
            h_new = gru_pool.tile([128, CW], BF16, tag=f"h{c}")
            nc.vector.tensor_tensor(h_new[:], zh[:], zn[:], ALU.subtract)
            return h_new

        # encoder
        hs = []
        for c in range(NCH):
            h0 = gru_pool.tile([128, CW], BF16, tag=f"h{c}", name=f"h0_{c}")
            nc.vector.memset(h0[:], 0.0)
            hs.append(h0)
        enc_xs = xt + [embs_t, embd_t]
        enc_pre = encpre_t[:] if enc_preload else None
        for t in range(T + 2):
            for c in range(NCH):
                hs[c] = gru_step(hs[c], enc_xs[t], c, whhTe_t, wihTe_t,
                                 enc_pre, binE_t[:, 0:1], bhnE_t[:, 0:1],
                                 enc=True)

        # decoder
        hd = hd_ps.tile([T, BS], F32, tag="hd")
        for t in range(T):
            for c in range(NCH):
                hs[c] = gru_step(hs[c], None, c, whhTd_t, None,
                                 decpre_t[:, t * E:(t + 1) * E],
                                 btD_t[:, t:t + 1], mscD_t[:, 0:1], enc=False)
                nc.tensor.matmul(
                    hd[:, c * CW:(c + 1) * CW],
                    onehot_t[:, t * 16:(t + 1) * 16], hs[c][:],
                    start=(t == 0 and c == 0), stop=(t == T - 1 and c == NCH - 1),
                    skip_group_check=True,
                )

        # ---------------- hdot drain: [t,b] -> [b,t] ----------------
        nc.vector.tensor_scalar(hd1_t[:], hd[:], mlpb16_t[:, 0:1], None, ALU.add)
        hd1b = pt("hd1b", [T, BS], BF16)
        nc.vector.tensor_copy(hd1b[:], hd1_t[:])
        hdp = g_ps.tile([BS, T], BF16, tag="gps", name="hdp")
        nc.tensor.transpose(hdp[:], hd1b[:], eye_t[0:T, 0:T])
        nc.vector.tensor_copy(hdot_t[:], hdp[:])

        # ---------------- softmax (t-independent) ----------------
        mx = pt("mx", [BS, 1])
        nc.vector.reduce_max(mx[:], nsproj_t[:], mybir.AxisListType.X)
        negmx = pt("negmx", [BS, 1])
        nc.vector.tensor_scalar(negmx[:], mx[:], -1.0, None, ALU.mult)
        ex = pt("ex", [BS, N])
        ssum = pt("ssum", [BS, 1])
        nc.scalar.activation(ex[:], nsproj_t[:], AF.Exp, bias=negmx[:, 0:1],
                             accum_out=ssum[:, 0:1])
        rsum = pt("rsum", [BS, 1])
        nc.vector.reciprocal(rsum[:], ssum[:])

        for t in range(T):
            nc.gpsimd.tensor_scalar(
                soft_all[:, t * N:(t + 1) * N], ex[:], rsum[:, 0:1], None,
                ALU.mult
            )
            nc.gpsimd.tensor_scalar(
                logits_all[:, t * N:(t + 1) * N], nsproj_t[:],
                hdot_t[:, t:t + 1], None, ALU.add
            )

        nc.sync.dma_start(out=logits_o[:], in_=logits_all[:])
        nc.sync.dma_start(out=soft_o[:], in_=soft_all[:])

    nc.finalize()  # Bacc.compile(): wait-splitting, reg alloc, nop fusion
    return nc


def _prep_inputs(inputs):
    """Host-side preprocessing -> per-core input maps + program flags."""
    f32 = np.float32
    bf16 = ml_dtypes.bfloat16
    net_x = np.ascontiguousarray(np.asarray(inputs["net_x"], dtype=f32))
    sfc_state = np.ascontiguousarray(np.asarray(inputs["sfc_state"], dtype=f32))
    edge_index = np.asarray(inputs["edge_index"]).astype(np.int64)
    source_dest = np.asarray(inputs["source_dest"]).astype(np.int64)
    node_embed = np.asarray(inputs["node_embed"], dtype=f32)
    gcn_w = np.asarray(inputs["gcn_w"], dtype=f32)
    gcn_b = np.asarray(inputs["gcn_b"], dtype=f32)
    enc_wih = np.asarray(inputs["enc_wih"], dtype=f32)
    enc_whh = np.asarray(inputs["enc_whh"], dtype=f32)
    enc_bih = np.asarray(inputs["enc_bih"], dtype=f32)
    enc_bhh = np.asarray(inputs["enc_bhh"], dtype=f32)
    dec_wih = np.asarray(inputs["dec_wih"], dtype=f32)
    dec_whh = np.asarray(inputs["dec_whh"], dtype=f32)
    dec_bih = np.asarray(inputs["dec_bih"], dtype=f32)
    dec_bhh = np.asarray(inputs["dec_bhh"], dtype=f32)
    mlp_w = np.asarray(inputs["mlp_w"], dtype=f32)
    mlp_b = np.asarray(inputs["mlp_b"], dtype=f32)

    # normalized adjacency with self-loops, transposed: AT[s, d]
    src = np.concatenate([edge_index[0], np.arange(N, dtype=np.int64)])
    dst = np.concatenate([edge_index[1], np.arange(N, dtype=np.int64)])
    deg = np.zeros(N, dtype=f32)
    np.add.at(deg, dst, f32(1.0))
    with np.errstate(divide="ignore"):
        dinv = (1.0 / np.sqrt(deg)).astype(f32)
    norm = (dinv[src] * dinv[dst]).astype(f32)
    AT = np.zeros((N, N), dtype=f32)
    np.add.at(AT, (src, dst), norm)

    embs_full = np.ascontiguousarray(node_embed[source_dest[:, 0]].T)  # [E, B]
    embd_full = np.ascontiguousarray(node_embed[source_dest[:, 1]].T)

    brE = enc_bih[0:E] + enc_bhh[0:E]
    bzE = enc_bih[E:2 * E] + enc_bhh[E:2 * E]
    enc_preload = bool(np.any(brE) or np.any(bzE))
    encpre = np.stack([brE, bzE]).astype(bf16)  # [2, E]

    gi_dec = node_embed[:T] @ dec_wih.T + dec_bih  # [T, 384]
    brD = gi_dec[:, 0:E] + dec_bhh[0:E]            # [T, E]
    bzD = gi_dec[:, E:2 * E] + dec_bhh[E:2 * E]
    decpre = np.zeros((2, T * E), dtype=bf16)
    for t in range(T):
        decpre[0, t * E:(t + 1) * E] = brD[t].astype(bf16)
        decpre[1, t * E:(t + 1) * E] = bzD[t].astype(bf16)
    btD = np.ascontiguousarray(gi_dec[:, 2 * E:3 * E].T)   # [E, T]
    mscD = np.ascontiguousarray(dec_bhh[2 * E:3 * E, None])

    selmask = np.zeros((2, 2 * CW), dtype=bf16)
    selmask[0, 0:CW] = 1
    selmask[1, CW:2 * CW] = 1

    mlp_w_bf = mlp_w.astype(bf16)
    onehot = np.zeros((E, 16 * T), dtype=bf16)
    for t in range(T):
        onehot[:, t * 16 + t] = mlp_w_bf
    ohproj = np.zeros((E, 32 * 32), dtype=bf16)
    for g in range(32):
        ohproj[:, g * 32 + g] = mlp_w_bf

    shared = {
        "at": AT.astype(bf16),
        "gcnw": np.vstack([gcn_w, gcn_w]).astype(bf16),
        "gcnb": np.ascontiguousarray(gcn_b[:, None]),
        "onehot": onehot,
        "ohproj": ohproj,
        "wihTe": np.ascontiguousarray(enc_wih.T).astype(bf16),
        "whhTe": np.ascontiguousarray(enc_whh.T).astype(bf16),
        "whhTd": np.ascontiguousarray(dec_whh.T).astype(bf16),
        "encpre": encpre,
        "decpre": decpre,
        "selmask": selmask,
        "binE": np.ascontiguousarray(enc_bih[2 * E:3 * E, None]),
        "bhnE": np.ascontiguousarray(enc_bhh[2 * E:3 * E, None]),
        "btD": btD,
        "mscD": mscD,
        "mlpb16": np.full((T, 1), float(mlp_b), dtype=f32),
        "eye": np.eye(128, dtype=bf16),
    }
    in_maps = []
    for c in range(NCORES):
        b0 = c * BS
        m = dict(shared)
        m["netx"] = np.ascontiguousarray(net_x[b0:b0 + BS])
        m["sfc"] = np.ascontiguousarray(sfc_state[b0:b0 + BS])
        m["embs"] = np.ascontiguousarray(embs_full[:, b0:b0 + BS])
        m["embd"] = np.ascontiguousarray(embd_full[:, b0:b0 + BS])
        in_maps.append(m)
    return in_maps, enc_preload


def _get_program(enc_preload: bool):
    key = ("v2", enc_preload)
    if key not in _CACHE:
        _CACHE[key] = _build_program(enc_preload)
    return _CACHE[key]


def run(inputs, trace=False, tmpdir=None):
    in_maps, enc_preload = _prep_inputs(inputs)
    nc = _get_program(enc_preload)
    res = run_bass_kernel_spmd(
        nc, in_maps, list(range(NCORES)), trace=trace, tmpdir=tmpdir
    )
    logits = np.concatenate(
        [res.results[c]["logits"].reshape(BS, T, N) for c in range(NCORES)], axis=0
    )
    soft = np.concatenate(
        [res.results[c]["soft"].reshape(BS, T, N) for c in range(NCORES)], axis=0
    )
    return (logits, soft), res


def kernel(**inputs):
    (logits, soft), _ = run(inputs, trace=False)
    return logits, soft


# revision 21
# speedup vs baseline: 1.4799x; 1.3326x over previous
"""Trainium2 Bass kernel for nn_DecoderActor (GCN + GRU encoder/decoder + softmax).

Math (reference):
  net_state = relu(A @ net_x_b @ gcn_w + gcn_b)            per batch b; A shared
  hidden    = GRU_enc over [sfc_state(16); emb(src); emb(dst)]  (18 steps)
  h_t       = GRU_dec(h_{t-1}, node_embed[t])              (16 steps, x batch-indep)
  logits[b,t,n] = ns_proj[b,n] + hdot[b,t] + mlp_b
      where ns_proj[b,n] = net_state[b,n,:] @ mlp_w, hdot[b,t] = h_t[b] @ mlp_w
  softmax over n == softmax(ns_proj) (the (b,t)-constant shift cancels exactly)

Sharding: pure data parallel, batch 1024 -> 8 cores x 128.

Device design (v2):
  - All matmuls bf16 (fp32 PSUM accumulation); fp32->bf16 casts ride gpsimd
    cast-DMAs and psum->sbuf copies, costing nothing extra.
  - GRU state h kept transposed [E=128 partitions, b free]; batch split in two
    64-wide chunks that pipeline through PE/ACT/DVE/GPSIMD to hide the serial
    per-step latency (sem hops) of the recurrence.
  - Gate pre-activations per chunk in one PSUM bank [128, 256]:
      cols 0:64 r | 64:128 z | 128:192 h_n | 192:256 i_n(enc only)
    x-side biases enter via a K<=2 "preload" matmul (bias rows x select mask),
    so sigmoid(r|z) is ONE ACT op with no bias.
  - GCN: P1 per batch-pair (stationary = natural-layout net_x pair, moving =
    A.T), P2 = gcn_w.T @ V.T at N=512, relu+bias fused into the psum->sbuf
    cast, ns_proj via one-hot stationaries accumulating all 16K values into a
    single PSUM bank, drained by one copy + one SBUF rearrange DMA.
"""

import numpy as np
import ml_dtypes

import concourse.bass as bass
import concourse.bacc as bacc
import concourse.tile as tile
from concourse import mybir
from concourse.bass_utils import run_bass_kernel_spmd

F32 = mybir.dt.float32
BF16 = mybir.dt.bfloat16
AF = mybir.ActivationFunctionType
ALU = mybir.AluOpType

B, N, T, E, F_NET, NUM_NODES = 1024, 128, 16, 128, 64, 128
NCORES = 8
BS = B // NCORES     # 128 batch per core
NPAIR = BS // 2      # 64 pairs
NTILE = NPAIR // 4   # 16 pair-tiles
CW = 64              # GRU chunk width
NCH = BS // CW       # 2 chunks

_CACHE = {}


def _build_program(enc_preload: bool):
    nc = bacc.Bacc(
        "TRN2",
        target_bir_lowering=False,
        debug=False,
        num_swdge_queues=4,
    )

    # ---------------- DRAM I/O ----------------
    def din(name, shape, dt=F32):
        return nc.dram_tensor(name, list(shape), dt, kind="ExternalInput").ap()

    netx_d = din("netx", [BS, N, F_NET])            # [b, s, f] f32
    sfc_d = din("sfc", [BS, T, E])                  # [b, t, e] f32
    at_d = din("at", [N, N], BF16)                  # A.T  [s, d]
    gcnw_d = din("gcnw", [2 * F_NET, E], BF16)      # gcn_w stacked twice
    gcnb_d = din("gcnb", [E, 1])
    onehot_d = din("onehot", [E, 16 * T], BF16)     # col t*16+t = mlp_w
    ohproj_d = din("ohproj", [E, 32 * 32], BF16)    # stat g: col g*32+g = mlp_w
    wihTe_d = din("wihTe", [E, 3 * E], BF16)        # enc_wih.T
    whhTe_d = din("whhTe", [E, 3 * E], BF16)        # enc_whh.T
    whhTd_d = din("whhTd", [E, 3 * E], BF16)        # dec_whh.T
    embs_d = din("embs", [E, BS])                   # emb(src).T slice, f32
    embd_d = din("embd", [E, BS])
    encpre_d = din("encpre", [2, E], BF16)          # [brE | bzE] rows
    decpre_d = din("decpre", [2, T * E], BF16)      # per-step [brD | bzD]
    selmask_d = din("selmask", [2, 2 * CW], BF16)   # row0: ones 0:CW; row1: CW:2CW
    binE_d = din("binE", [E, 1])
    bhnE_d = din("bhnE", [E, 1])
    btD_d = din("btD", [E, T])                      # dec tanh bias per step
    mscD_d = din("mscD", [E, 1])                    # dec_bhh_n
    mlpb16_d = din("mlpb16", [T, 1])                # mlp_b replicated
    eye_d = din("eye", [128, 128], BF16)

    logits_o = nc.dram_tensor("logits", [BS, T * N], F32, kind="ExternalOutput").ap()
    soft_o = nc.dram_tensor("soft", [BS, T * N], F32, kind="ExternalOutput").ap()

    from contextlib import ExitStack

    with tile.TileContext(nc) as tc, ExitStack() as ctx:
        # ---------------- pools ----------------
        persist = ctx.enter_context(tc.tile_pool(name="persist", bufs=1))
        vt_pool = ctx.enter_context(tc.tile_pool(name="vt", bufs=3))
        nst_pool = ctx.enter_context(tc.tile_pool(name="nst", bufs=4))
        gru_pool = ctx.enter_context(tc.tile_pool(name="gru", bufs=3))
        # psum banks: gps(3) + p1(2) + p2(1) + proj(1) + hd(1) = 8
        g_ps = ctx.enter_context(tc.tile_pool(name="gps", bufs=3, space="PSUM"))
        p1_ps = ctx.enter_context(tc.tile_pool(name="p1ps", bufs=2, space="PSUM"))
        p2_ps = ctx.enter_context(tc.tile_pool(name="p2ps", bufs=1, space="PSUM"))
        pr_ps = ctx.enter_context(tc.tile_pool(name="prps", bufs=1, space="PSUM"))
        hd_ps = ctx.enter_context(tc.tile_pool(name="hdps", bufs=1, space="PSUM"))

        def pt(name, shape, dt=F32):
            return persist.tile(list(shape), dt, tag=name, name=name)

        # ---------------- const tiles + loads ----------------
        at_t = pt("at", [N, N], BF16)
        gcnw_t = pt("gcnw", [2 * F_NET, E], BF16)
        gcnb_t = pt("gcnb", [E, 1])
        onehot_t = pt("onehot", [E, 16 * T], BF16)
        ohproj_t = pt("ohproj", [E, 32 * 32], BF16)
        prsb_t = pt("prsb", [32, 512])
        wihTe_t = pt("wihTe", [E, 3 * E], BF16)
        whhTe_t = pt("whhTe", [E, 3 * E], BF16)
        whhTd_t = pt("whhTd", [E, 3 * E], BF16)
        encpre_t = pt("encpre", [2, E], BF16)
        decpre_t = pt("decpre", [2, T * E], BF16)
        selmask_t = pt("selmask", [2, 2 * CW], BF16)
        binE_t = pt("binE", [E, 1])
        bhnE_t = pt("bhnE", [E, 1])
        btD_t = pt("btD", [E, T])
        mscD_t = pt("mscD", [E, 1])
        mlpb16_t = pt("mlpb16", [T, 1])
        eye_t = pt("eye", [128, 128], BF16)
        nsproj_t = pt("nsproj", [BS, N])
        hdot_t = pt("hdot", [BS, T])
        hd1_t = pt("hd1", [T, BS])
        logits_all = pt("logits_all", [BS, T * N])
        soft_all = pt("soft_all", [BS, T * N])
        nxall = pt("nxall", [N, NPAIR * 128], BF16)   # all pairs, pair p cols p*128
        sfall = pt("sfall", [BS, T * E], BF16)
        xtall = pt("xtall", [E, (T + 2) * BS], BF16)

        for tl, d in [
            (at_t, at_d), (gcnw_t, gcnw_d), (gcnb_t, gcnb_d),
            (onehot_t, onehot_d), (ohproj_t, ohproj_d), (wihTe_t, wihTe_d),
            (whhTe_t, whhTe_d), (whhTd_t, whhTd_d), (encpre_t, encpre_d),
            (decpre_t, decpre_d), (selmask_t, selmask_d), (binE_t, binE_d),
            (bhnE_t, bhnE_d), (btD_t, btD_d), (mscD_t, mscD_d),
            (mlpb16_t, mlpb16_d), (eye_t, eye_d),
        ]:
            nc.sync.dma_start(out=tl[:], in_=d[:])

        # ---------------- bulk cast DMAs (gpsimd = cast-capable) ----------
        # sfc: 4 DMAs of 4 t each
        for q in range(4):
            nc.gpsimd.dma_start(
                out=sfall[:, q * 4 * E:(q + 1) * 4 * E],
                in_=sfc_d[:, q * 4:(q + 1) * 4, :],
            )
        # net_x: 8 DMAs; group g covers pairs 16g..16g+15
        nx_r = nxall[:].rearrange("s (p c) -> s p c", p=NPAIR)
        for gdx in range(4):
            p0 = 16 * gdx
            nc.gpsimd.dma_start(
                out=nx_r[:, p0:p0 + 16, 0:F_NET],
                in_=netx_d[p0:p0 + 16].rearrange("b s f -> s b f"),
            )
            nc.gpsimd.dma_start(
                out=nx_r[:, p0:p0 + 16, F_NET:128],
                in_=netx_d[NPAIR + p0:NPAIR + p0 + 16].rearrange("b s f -> s b f"),
            )
        # emb columns -> xtall steps 16, 17
        nc.gpsimd.dma_start(out=xtall[:, 16 * BS:17 * BS], in_=embs_d[:])
        nc.gpsimd.dma_start(out=xtall[:, 17 * BS:18 * BS], in_=embd_d[:])

        # ---------------- sfc PE transposes ----------------
        for t in range(T):
            tp = g_ps.tile([E, BS], BF16, tag="gps", name=f"tp{t}")
            nc.tensor.transpose(tp[:], sfall[:, t * E:(t + 1) * E], eye_t[:])
            if t % 2 == 0:
                nc.vector.tensor_copy(xtall[:, t * BS:(t + 1) * BS], tp[:])
            else:
                nc.scalar.copy(xtall[:, t * BS:(t + 1) * BS], tp[:])

        prp = pr_ps.tile([128, 512], F32, tag="proj", name="prp")

        def gcn_tile(j):
            """One GCN pair-tile: 4x P1, vt copy, 2x (P2 + relu + proj)."""
            p1 = p1_ps.tile([128, 512], F32, tag="p1", name=f"p1_{j}")
            for k in range(4):
                c0 = (4 * j + k) * 128
                nc.tensor.matmul(
                    p1[:, k * 128:(k + 1) * 128],
                    nxall[:, c0:c0 + 128], at_t[:],
                    start=(k == 0), stop=(k == 3), skip_group_check=True,
                )
            vt = vt_pool.tile([128, 512], BF16, tag="vt")
            if j % 2 == 0:
                nc.vector.tensor_copy(vt[:], p1[:])
            else:
                nc.scalar.copy(vt[:], p1[:])
            for half in range(2):
                p2 = p2_ps.tile([E, 512], F32, tag="p2", name=f"p2_{j}_{half}")
                nc.tensor.matmul(
                    p2[:], gcnw_t[64 * half:64 * (half + 1), :],
                    vt[64 * half:64 * (half + 1), :],
                    start=True, stop=True,
                )
                nst = nst_pool.tile([E, 512], BF16, tag="nst")
                if (2 * j + half) % 2 == 0:
                    nc.scalar.activation(nst[:], p2[:], AF.Relu, bias=gcnb_t[:, 0:1])
                else:
                    nc.vector.tensor_scalar(
                        nst[:], p2[:], gcnb_t[:, 0:1], 0.0, ALU.add, ALU.max
                    )
                g = j + 16 * half
                nc.tensor.matmul(
                    prp[0:32, :], ohproj_t[:, g * 32:(g + 1) * 32], nst[:],
                    start=(g == 0), stop=(g == 31), skip_group_check=True,
                )

        # ---------------- GRU: both chunks emitted stage-interleaved -------
        def gru_step_both(hs, xt_col, wT_h, wT_x, pre_stat, tanh_bias, m_scalar,
                          enc: bool):
            Ps, rzs, ms, ss, ns_, zhs, zns, hnew = [], [], [], [], [], [], [], []
            for c in range(NCH):
                P = g_ps.tile([128, 4 * CW], F32, tag="gps", name=None)
                Ps.append(P)
            use_pre = pre_stat is not None
            for c in range(NCH):
                if use_pre:
                    nc.tensor.matmul(Ps[c][:, 0:2 * CW], pre_stat, selmask_t[:],
                                     start=True, stop=False, skip_group_check=True)
            for c in range(NCH):
                nc.tensor.matmul(Ps[c][:, 0:CW], wT_h[:, 0:128], hs[c][:],
                                 start=not use_pre, stop=False,
                                 skip_group_check=True)
            for c in range(NCH):
                nc.tensor.matmul(Ps[c][:, CW:2 * CW], wT_h[:, 128:256], hs[c][:],
                                 start=False, stop=False, skip_group_check=True)
            for c in range(NCH):
                nc.tensor.matmul(Ps[c][:, 2 * CW:3 * CW], wT_h[:, 256:384],
                                 hs[c][:], start=False, stop=not enc,
                                 skip_group_check=True)
            if enc:
                for c in range(NCH):
                    lo = xt_col + c * CW
                    nc.tensor.matmul(Ps[c][:, 0:CW], wT_x[:, 0:128],
                                     xtall[:, lo:lo + CW], start=False,
                                     stop=False, skip_group_check=True)
                for c in range(NCH):
                    lo = xt_col + c * CW
                    nc.tensor.matmul(Ps[c][:, CW:2 * CW], wT_x[:, 128:256],
                                     xtall[:, lo:lo + CW], start=False,
                                     stop=False, skip_group_check=True)
                for c in range(NCH):
                    lo = xt_col + c * CW
                    nc.tensor.matmul(Ps[c][:, 3 * CW:4 * CW], wT_x[:, 256:384],
                                     xtall[:, lo:lo + CW], start=False,
                                     stop=True, skip_group_check=True)
            for c in range(NCH):
                rz = gru_pool.tile([128, 2 * CW], BF16, tag=f"rz{c}")
                nc.scalar.activation(rz[:], Ps[c][:, 0:2 * CW], AF.Sigmoid)
                rzs.append(rz)
            for c in range(NCH):
                m_t = gru_pool.tile([128, CW], F32, tag=f"m{c}")
                nc.vector.scalar_tensor_tensor(
                    m_t[:], Ps[c][:, 2 * CW:3 * CW], m_scalar, rzs[c][:, 0:CW],
                    ALU.add, ALU.mult
                )
                ms.append(m_t)
            if enc:
                for c in range(NCH):
                    s_t = gru_pool.tile([128, CW], F32, tag=f"s{c}")
                    nc.vector.tensor_tensor(s_t[:], ms[c][:],
                                            Ps[c][:, 3 * CW:4 * CW], ALU.add)
                    ss.append(s_t)
            else:
                ss = ms
            for c in range(NCH):
                n_t = gru_pool.tile([128, CW], BF16, tag=f"n{c}")
                nc.scalar.activation(n_t[:], ss[c][:], AF.Tanh, bias=tanh_bias)
                ns_.append(n_t)
            for c in range(NCH):
                zh = gru_pool.tile([128, CW], BF16, tag=f"zh{c}")
                nc.gpsimd.tensor_tensor(zh[:], rzs[c][:, CW:2 * CW], hs[c][:],
                                        ALU.mult)
                zhs.append(zh)
            for c in range(NCH):
                zn = gru_pool.tile([128, CW], BF16, tag=f"zn{c}")
                nc.vector.scalar_tensor_tensor(
                    zn[:], rzs[c][:, CW:2 * CW], 1.0, ns_[c][:],
                    ALU.subtract, ALU.mult
                )
                zns.append(zn)
            for c in range(NCH):
                h_new = gru_pool.tile([128, CW], BF16, tag=f"h{c}")
                nc.vector.tensor_tensor(h_new[:], zhs[c][:], zns[c][:],
                                        ALU.subtract)
                hnew.append(h_new)
            return hnew

        # encoder, with one GCN tile interleaved per step
        hs = []
        for c in range(NCH):
            h0 = gru_pool.tile([128, CW], BF16, tag=f"h{c}", name=f"h0_{c}")
            nc.vector.memset(h0[:], 0.0)
            hs.append(h0)
        enc_pre = encpre_t[:] if enc_preload else None
        for t in range(T + 2):
            hs = gru_step_both(hs, t * BS, whhTe_t, wihTe_t, enc_pre,
                               binE_t[:, 0:1], bhnE_t[:, 0:1], enc=True)
            if t < NTILE:
                gcn_tile(t)

        # drain ns_proj: psum rows g -> sbuf -> rearrange DMA to [b, d]
        nc.vector.tensor_copy(prsb_t[:], prp[0:32, :])
        nc.sync.dma_start(out=nsproj_t[:], in_=prsb_t[:])

        # softmax pieces (t-independent; emitted before decoder so the ops
        # fill decode-phase gaps)
        mx = pt("mx", [BS, 1])
        nc.vector.reduce_max(mx[:], nsproj_t[:], mybir.AxisListType.X)
        negmx = pt("negmx", [BS, 1])
        nc.vector.tensor_scalar(negmx[:], mx[:], -1.0, None, ALU.mult)
        ex = pt("ex", [BS, N])
        ssum = pt("ssum", [BS, 1])
        nc.scalar.activation(ex[:], nsproj_t[:], AF.Exp, bias=negmx[:, 0:1],
                             accum_out=ssum[:, 0:1])
        rsum = pt("rsum", [BS, 1])
        nc.vector.reciprocal(rsum[:], ssum[:])

        # decoder, with softmax slice writes + DMAs interleaved
        hd = hd_ps.tile([T, BS], F32, tag="hd")
        for t in range(T):
            hs = gru_step_both(hs, 0, whhTd_t, None,
                               decpre_t[:, t * E:(t + 1) * E],
                               btD_t[:, t:t + 1], mscD_t[:, 0:1], enc=False)
            for c in range(NCH):
                nc.tensor.matmul(
                    hd[:, c * CW:(c + 1) * CW],
                    onehot_t[:, t * 16:(t + 1) * 16], hs[c][:],
                    start=(t == 0 and c == 0),
                    stop=(t == T - 1 and c == NCH - 1),
                    skip_group_check=True,
                )
            nc.vector.tensor_scalar(
                soft_all[:, t * N:(t + 1) * N], ex[:], rsum[:, 0:1], None,
                ALU.mult
            )
            nc.sync.dma_start(out=soft_o[:, t * N:(t + 1) * N],
                              in_=soft_all[:, t * N:(t + 1) * N])

        # ---------------- hdot drain: [t,b] -> [b,t] ----------------
        nc.vector.tensor_scalar(hd1_t[:], hd[:], mlpb16_t[:, 0:1], None, ALU.add)
        hd1b = pt("hd1b", [T, BS], BF16)
        nc.vector.tensor_copy(hd1b[:], hd1_t[:])
        hdp = g_ps.tile([BS, T], BF16, tag="gps", name="hdp")
        nc.tensor.transpose(hdp[:], hd1b[:], eye_t[0:T, 0:T])
        nc.vector.tensor_copy(hdot_t[:], hdp[:])

        for t in range(T):
            nc.vector.tensor_scalar(
                logits_all[:, t * N:(t + 1) * N], nsproj_t[:],
                hdot_t[:, t:t + 1], None, ALU.add
            )
            nc.sync.dma_start(out=logits_o[:, t * N:(t + 1) * N],
                              in_=logits_all[:, t * N:(t + 1) * N])

    nc.finalize()  # Bacc.compile(): wait-splitting, reg alloc, nop fusion
    return nc


def _prep_inputs(inputs):
    """Host-side preprocessing -> per-core input maps + program flags."""
    f32 = np.float32
    bf16 = ml_dtypes.bfloat16
    net_x = np.ascontiguousarray(np.asarray(inputs["net_x"], dtype=f32))
    sfc_state = np.ascontiguousarray(np.asarray(inputs["sfc_state"], dtype=f32))
    edge_index = np.asarray(inputs["edge_index"]).astype(np.int64)
    source_dest = np.asarray(inputs["source_dest"]).astype(np.int64)
    node_embed = np.asarray(inputs["node_embed"], dtype=f32)
    gcn_w = np.asarray(inputs["gcn_w"], dtype=f32)
    gcn_b = np.asarray(inputs["gcn_b"], dtype=f32)
    enc_wih = np.asarray(inputs["enc_wih"], dtype=f32)
    enc_whh = np.asarray(inputs["enc_whh"], dtype=f32)
    enc_bih = np.asarray(inputs["enc_bih"], dtype=f32)
    enc_bhh = np.asarray(inputs["enc_bhh"], dtype=f32)
    dec_wih = np.asarray(inputs["dec_wih"], dtype=f32)
    dec_whh = np.asarray(inputs["dec_whh"], dtype=f32)
    dec_bih = np.asarray(inputs["dec_bih"], dtype=f32)
    dec_bhh = np.asarray(inputs["dec_bhh"], dtype=f32)
    mlp_w = np.asarray(inputs["mlp_w"], dtype=f32)
    mlp_b = np.asarray(inputs["mlp_b"], dtype=f32)

    # normalized adjacency with self-loops, transposed: AT[s, d]
    src = np.concatenate([edge_index[0], np.arange(N, dtype=np.int64)])
    dst = np.concatenate([edge_index[1], np.arange(N, dtype=np.int64)])
    deg = np.zeros(N, dtype=f32)
    np.add.at(deg, dst, f32(1.0))
    with np.errstate(divide="ignore"):
        dinv = (1.0 / np.sqrt(deg)).astype(f32)
    norm = (dinv[src] * dinv[dst]).astype(f32)
    AT = np.zeros((N, N), dtype=f32)
    np.add.at(AT, (src, dst), norm)

    embs_full = np.ascontiguousarray(node_embed[source_dest[:, 0]].T)  # [E, B]
    embd_full = np.ascontiguousarray(node_embed[source_dest[:, 1]].T)

    brE = enc_bih[0:E] + enc_bhh[0:E]
    bzE = enc_bih[E:2 * E] + enc_bhh[E:2 * E]
    enc_preload = bool(np.any(brE) or np.any(bzE))
    encpre = np.stack([brE, bzE]).astype(bf16)  # [2, E]

    gi_dec = node_embed[:T] @ dec_wih.T + dec_bih  # [T, 384]
    brD = gi_dec[:, 0:E] + dec_bhh[0:E]            # [T, E]
    bzD = gi_dec[:, E:2 * E] + dec_bhh[E:2 * E]
    decpre = np.zeros((2, T * E), dtype=bf16)
    for t in range(T):
        decpre[0, t * E:(t + 1) * E] = brD[t].astype(bf16)
        decpre[1, t * E:(t + 1) * E] = bzD[t].astype(bf16)
    btD = np.ascontiguousarray(gi_dec[:, 2 * E:3 * E].T)   # [E, T]
    mscD = np.ascontiguousarray(dec_bhh[2 * E:3 * E, None])

    selmask = np.zeros((2, 2 * CW), dtype=bf16)
    selmask[0, 0:CW] = 1
    selmask[1, CW:2 * CW] = 1

    mlp_w_bf = mlp_w.astype(bf16)
    onehot = np.zeros((E, 16 * T), dtype=bf16)
    for t in range(T):
        onehot[:, t * 16 + t] = mlp_w_bf
    ohproj = np.zeros((E, 32 * 32), dtype=bf16)
    for g in range(32):
        ohproj[:, g * 32 + g] = mlp_w_bf

    shared = {
        "at": AT.astype(bf16),
        "gcnw": np.vstack([gcn_w, gcn_w]).astype(bf16),
        "gcnb": np.ascontiguousarray(gcn_b[:, None]),
        "onehot": onehot,
        "ohproj": ohproj,
        "wihTe": np.ascontiguousarray(enc_wih.T).astype(bf16),
        "whhTe": np.ascontiguousarray(enc_whh.T).astype(bf16),
        "whhTd": np.ascontiguousarray(dec_whh.T).astype(bf16),
        "encpre": encpre,
        "decpre": decpre,
        "selmask": selmask,
        "binE": np.ascontiguousarray(enc_bih[2 * E:3 * E, None]),
        "bhnE": np.ascontiguousarray(enc_bhh[2 * E:3 * E, None]),
        "btD": btD,
        "mscD": mscD,
        "mlpb16": np.full((T, 1), float(mlp_b), dtype=f32),
        "eye": np.eye(128, dtype=bf16),
    }
    in_maps = []
    for c in range(NCORES):
        b0 = c * BS
        m = dict(shared)
        m["netx"] = np.ascontiguousarray(net_x[b0:b0 + BS])
        m["sfc"] = np.ascontiguousarray(sfc_state[b0:b0 + BS])
        m["embs"] = np.ascontiguousarray(embs_full[:, b0:b0 + BS])
        m["embd"] = np.ascontiguousarray(embd_full[:, b0:b0 + BS])
        in_maps.append(m)
    return in_maps, enc_preload


def _get_program(enc_preload: bool):
    key = ("v2", enc_preload)
    if key not in _CACHE:
        _CACHE[key] = _build_program(enc_preload)
    return _CACHE[key]


def run(inputs, trace=False, tmpdir=None):
    in_maps, enc_preload = _prep_inputs(inputs)
    nc = _get_program(enc_preload)
    res = run_bass_kernel_spmd(
        nc, in_maps, list(range(NCORES)), trace=trace, tmpdir=tmpdir
    )
    logits = np.concatenate(
        [res.results[c]["logits"].reshape(BS, T, N) for c in range(NCORES)], axis=0
    )
    soft = np.concatenate(
        [res.results[c]["soft"].reshape(BS, T, N) for c in range(NCORES)], axis=0
    )
    return (logits, soft), res


def kernel(**inputs):
    (logits, soft), _ = run(inputs, trace=False)
    return logits, soft


# revision 23
# speedup vs baseline: 1.5592x; 1.0536x over previous
"""Trainium2 Bass kernel for nn_DecoderActor (GCN + GRU encoder/decoder + softmax).

Math (reference):
  net_state = relu(A @ net_x_b @ gcn_w + gcn_b)            per batch b; A shared
  hidden    = GRU_enc over [sfc_state(16); emb(src); emb(dst)]  (18 steps)
  h_t       = GRU_dec(h_{t-1}, node_embed[t])              (16 steps, x batch-indep)
  logits[b,t,n] = ns_proj[b,n] + hdot[b,t] + mlp_b
      where ns_proj[b,n] = net_state[b,n,:] @ mlp_w, hdot[b,t] = h_t[b] @ mlp_w
  softmax over n == softmax(ns_proj) (the (b,t)-constant shift cancels exactly)

Sharding: pure data parallel, batch 1024 -> 8 cores x 128.

Device design (v2):
  - All matmuls bf16 (fp32 PSUM accumulation); fp32->bf16 casts ride gpsimd
    cast-DMAs and psum->sbuf copies, costing nothing extra.
  - GRU state h kept transposed [E=128 partitions, b free]; batch split in two
    64-wide chunks that pipeline through PE/ACT/DVE/GPSIMD to hide the serial
    per-step latency (sem hops) of the recurrence.
  - Gate pre-activations per chunk in one PSUM bank [128, 256]:
      cols 0:64 r | 64:128 z | 128:192 h_n | 192:256 i_n(enc only)
    x-side biases enter via a K<=2 "preload" matmul (bias rows x select mask),
    so sigmoid(r|z) is ONE ACT op with no bias.
  - GCN: P1 per batch-pair (stationary = natural-layout net_x pair, moving =
    A.T), P2 = gcn_w.T @ V.T at N=512, relu+bias fused into the psum->sbuf
    cast, ns_proj via one-hot stationaries accumulating all 16K values into a
    single PSUM bank, drained by one copy + one SBUF rearrange DMA.
"""

import numpy as np
import ml_dtypes

import concourse.bass as bass
import concourse.bacc as bacc
import concourse.tile as tile
from concourse import mybir
from concourse.bass_utils import run_bass_kernel_spmd

F32 = mybir.dt.float32
BF16 = mybir.dt.bfloat16
AF = mybir.ActivationFunctionType
ALU = mybir.AluOpType

B, N, T, E, F_NET, NUM_NODES = 1024, 128, 16, 128, 64, 128
NCORES = 8
BS = B // NCORES     # 128 batch per core
NPAIR = BS // 2      # 64 pairs
NTILE = NPAIR // 4   # 16 pair-tiles
CW = 64              # GRU chunk width
NCH = BS // CW       # 2 chunks

_CACHE = {}


def _build_program(enc_preload: bool):
    nc = bacc.Bacc(
        "TRN2",
        target_bir_lowering=False,
        debug=False,
        num_swdge_queues=4,
    )

    # ---------------- DRAM I/O ----------------
    def din(name, shape, dt=F32):
        return nc.dram_tensor(name, list(shape), dt, kind="ExternalInput").ap()

    netx_d = din("netx", [BS, N, F_NET])            # [b, s, f] f32
    sfc_d = din("sfc", [BS, T, E])                  # [b, t, e] f32
    at_d = din("at", [N, N])                  # A.T  [s, d]
    gcnw_d = din("gcnw", [2 * F_NET, E], BF16)      # gcn_w stacked twice
    gcnb_d = din("gcnb", [E, 1])
    onehot_d = din("onehot", [E, 16 * T], BF16)     # col t*16+t = mlp_w
    ohproj_d = din("ohproj", [E, 32 * 32], BF16)    # stat g: col g*32+g = mlp_w
    wihTe_d = din("wihTe", [E, 3 * E], BF16)        # enc_wih.T
    whhTe_d = din("whhTe", [E, 3 * E], BF16)        # enc_whh.T
    whhTd_d = din("whhTd", [E, 3 * E], BF16)        # dec_whh.T
    embs_d = din("embs", [E, BS])                   # emb(src).T slice, f32
    embd_d = din("embd", [E, BS])
    encpre_d = din("encpre", [2, E], BF16)          # [brE | bzE] rows
    decpre_d = din("decpre", [2, T * E], BF16)      # per-step [brD | bzD]
    selmask_d = din("selmask", [2, 2 * CW], BF16)   # row0: ones 0:CW; row1: CW:2CW
    binE_d = din("binE", [E, 1])
    bhnE_d = din("bhnE", [E, 1])
    btD_d = din("btD", [E, T])                      # dec tanh bias per step
    mscD_d = din("mscD", [E, 1])                    # dec_bhh_n
    mlpb16_d = din("mlpb16", [T, 1])                # mlp_b replicated
    eye_d = din("eye", [128, 128])

    logits_o = nc.dram_tensor("logits", [BS, T * N], F32, kind="ExternalOutput").ap()
    soft_o = nc.dram_tensor("soft", [BS, T * N], F32, kind="ExternalOutput").ap()

    from contextlib import ExitStack

    with tile.TileContext(nc) as tc, ExitStack() as ctx:
        # ---------------- pools ----------------
        persist = ctx.enter_context(tc.tile_pool(name="persist", bufs=1))
        vt_pool = ctx.enter_context(tc.tile_pool(name="vt", bufs=3))
        nst_pool = ctx.enter_context(tc.tile_pool(name="nst", bufs=4))
        gru_pool = ctx.enter_context(tc.tile_pool(name="gru", bufs=3))
        # psum banks: gps(3) + p1(2) + p2(1) + proj(1) + hd(1) = 8
        g_ps = ctx.enter_context(tc.tile_pool(name="gps", bufs=3, space="PSUM"))
        p1_ps = ctx.enter_context(tc.tile_pool(name="p1ps", bufs=2, space="PSUM"))
        p2_ps = ctx.enter_context(tc.tile_pool(name="p2ps", bufs=1, space="PSUM"))
        pr_ps = ctx.enter_context(tc.tile_pool(name="prps", bufs=1, space="PSUM"))
        hd_ps = ctx.enter_context(tc.tile_pool(name="hdps", bufs=1, space="PSUM"))

        def pt(name, shape, dt=F32):
            return persist.tile(list(shape), dt, tag=name, name=name)

        # ---------------- const tiles ----------------
        at_t = pt("at", [N, N])
        gcnw_t = pt("gcnw", [2 * F_NET, E], BF16)
        gcnb_t = pt("gcnb", [E, 1])
        onehot_t = pt("onehot", [E, 16 * T], BF16)
        ohproj_t = pt("ohproj", [E, 32 * 32], BF16)
        prsb_t = pt("prsb", [32, 512])
        wihTe_t = pt("wihTe", [E, 3 * E], BF16)
        whhTe_t = pt("whhTe", [E, 3 * E], BF16)
        whhTd_t = pt("whhTd", [E, 3 * E], BF16)
        encpre_t = pt("encpre", [2, E], BF16)
        decpre_t = pt("decpre", [2, T * E], BF16)
        selmask_t = pt("selmask", [2, 2 * CW], BF16)
        binE_t = pt("binE", [E, 1])
        bhnE_t = pt("bhnE", [E, 1])
        btD_t = pt("btD", [E, T])
        mscD_t = pt("mscD", [E, 1])
        mlpb16_t = pt("mlpb16", [T, 1])
        eye_t = pt("eye", [128, 128])
        nsproj_t = pt("nsproj", [BS, N])
        hdot_t = pt("hdot", [BS, T])
        hd1_t = pt("hd1", [T, BS])
        logits_all = pt("logits_all", [BS, T * N])
        soft_all = pt("soft_all", [BS, T * N])
        nxall = pt("nxall", [N, NPAIR * 128])          # f32, pair p at cols p*128
        sfall = pt("sfall", [BS, T * E])               # f32
        xtall = pt("xtall", [E, (T + 2) * BS], BF16)

        # weights/eye first (needed immediately), then sfc, then netx groups
        for tl, d in [
            (eye_t, eye_d), (whhTe_t, whhTe_d), (wihTe_t, wihTe_d),
            (binE_t, binE_d), (bhnE_t, bhnE_d), (encpre_t, encpre_d),
            (selmask_t, selmask_d), (at_t, at_d), (gcnw_t, gcnw_d),
            (gcnb_t, gcnb_d), (ohproj_t, ohproj_d), (onehot_t, onehot_d),
            (whhTd_t, whhTd_d), (decpre_t, decpre_d), (btD_t, btD_d),
            (mscD_t, mscD_d), (mlpb16_t, mlpb16_d),
        ]:
            nc.sync.dma_start(out=tl[:], in_=d[:])

        # sfc: one contiguous f32 load (8KB per partition row)
        nc.sync.dma_start(out=sfall[:], in_=sfc_d[:].rearrange("b t e -> b (t e)"))
        # emb columns -> xtall steps 16, 17 (gpsimd cast f32->bf16)
        nc.gpsimd.dma_start(out=xtall[:, 16 * BS:17 * BS], in_=embs_d[:])
        nc.gpsimd.dma_start(out=xtall[:, 17 * BS:18 * BS], in_=embd_d[:])
        # net_x: strided gather f32, spread across 4 engine queues
        nx_r = nxall[:].rearrange("s (p c) -> s p c", p=NPAIR)
        dma_engs = [nc.sync, nc.scalar, nc.gpsimd]
        for gdx in range(4):
            p0 = 16 * gdx
            dma_engs[(2 * gdx) % 3].dma_start(
                out=nx_r[:, p0:p0 + 16, 0:F_NET],
                in_=netx_d[p0:p0 + 16].rearrange("b s f -> s b f"),
            )
            dma_engs[(2 * gdx + 1) % 3].dma_start(
                out=nx_r[:, p0:p0 + 16, F_NET:128],
                in_=netx_d[NPAIR + p0:NPAIR + p0 + 16].rearrange("b s f -> s b f"),
            )

        # ---------------- sfc PE transposes (fp32) ----------------
        for t in range(T):
            tp = g_ps.tile([E, BS], F32, tag="gps", name=f"tp{t}")
            nc.tensor.transpose(tp[:], sfall[:, t * E:(t + 1) * E], eye_t[:])
            if t % 2 == 0:
                nc.vector.tensor_copy(xtall[:, t * BS:(t + 1) * BS], tp[:])
            else:
                nc.scalar.copy(xtall[:, t * BS:(t + 1) * BS], tp[:])

        prp = pr_ps.tile([128, 512], F32, tag="proj", name="prp")

        def gcn_tile(j):
            """One GCN pair-tile: 4x P1 (fp32), vt copy, 2x (P2 + relu + proj)."""
            p1 = p1_ps.tile([128, 512], F32, tag="p1", name=f"p1_{j}")
            for k in range(4):
                c0 = (4 * j + k) * 128
                nc.tensor.matmul(
                    p1[:, k * 128:(k + 1) * 128],
                    nxall[:, c0:c0 + 128], at_t[:],
                    start=(k == 0), stop=(k == 3), skip_group_check=True,
                )
            vt = vt_pool.tile([128, 512], BF16, tag="vt")
            if j % 2 == 0:
                nc.vector.tensor_copy(vt[:], p1[:])
            else:
                nc.scalar.copy(vt[:], p1[:])
            for half in range(2):
                p2 = p2_ps.tile([E, 512], F32, tag="p2", name=f"p2_{j}_{half}")
                nc.tensor.matmul(
                    p2[:], gcnw_t[64 * half:64 * (half + 1), :],
                    vt[64 * half:64 * (half + 1), :],
                    start=True, stop=True,
                )
                nst = nst_pool.tile([E, 512], BF16, tag="nst")
                if (2 * j + half) % 2 == 0:
                    nc.scalar.activation(nst[:], p2[:], AF.Relu, bias=gcnb_t[:, 0:1])
                else:
                    nc.vector.tensor_scalar(
                        nst[:], p2[:], gcnb_t[:, 0:1], 0.0, ALU.add, ALU.max
                    )
                g = j + 16 * half
                nc.tensor.matmul(
                    prp[0:32, :], ohproj_t[:, g * 32:(g + 1) * 32], nst[:],
                    start=(g == 0), stop=(g == 31), skip_group_check=True,
                )

        # ---------------- GRU: both chunks emitted stage-interleaved -------
        def gru_step_both(hs, xt_col, wT_h, wT_x, pre_stat, tanh_bias, m_scalar,
                          enc: bool):
            Ps, rzs, ms, ss, ns_, zhs, zns, hnew = [], [], [], [], [], [], [], []
            for c in range(NCH):
                P = g_ps.tile([128, 4 * CW], F32, tag="gps", name=None)
                Ps.append(P)
            use_pre = pre_stat is not None
            for c in range(NCH):
                if use_pre:
                    nc.tensor.matmul(Ps[c][:, 0:2 * CW], pre_stat, selmask_t[:],
                                     start=True, stop=False, skip_group_check=True)
            for c in range(NCH):
                nc.tensor.matmul(Ps[c][:, 0:CW], wT_h[:, 0:128], hs[c][:],
                                 start=not use_pre, stop=False,
                                 skip_group_check=True)
            for c in range(NCH):
                nc.tensor.matmul(Ps[c][:, CW:2 * CW], wT_h[:, 128:256], hs[c][:],
                                 start=False, stop=False, skip_group_check=True)
            for c in range(NCH):
                nc.tensor.matmul(Ps[c][:, 2 * CW:3 * CW], wT_h[:, 256:384],
                                 hs[c][:], start=False, stop=not enc,
                                 skip_group_check=True)
            if enc:
                for c in range(NCH):
                    lo = xt_col + c * CW
                    nc.tensor.matmul(Ps[c][:, 0:CW], wT_x[:, 0:128],
                                     xtall[:, lo:lo + CW], start=False,
                                     stop=False, skip_group_check=True)
                for c in range(NCH):
                    lo = xt_col + c * CW
                    nc.tensor.matmul(Ps[c][:, CW:2 * CW], wT_x[:, 128:256],
                                     xtall[:, lo:lo + CW], start=False,
                                     stop=False, skip_group_check=True)
                for c in range(NCH):
                    lo = xt_col + c * CW
                    nc.tensor.matmul(Ps[c][:, 3 * CW:4 * CW], wT_x[:, 256:384],
                                     xtall[:, lo:lo + CW], start=False,
                                     stop=True, skip_group_check=True)
            for c in range(NCH):
                rz = gru_pool.tile([128, 2 * CW], BF16, tag=f"rz{c}")
                nc.scalar.activation(rz[:], Ps[c][:, 0:2 * CW], AF.Sigmoid)
                rzs.append(rz)
            for c in range(NCH):
                m_t = gru_pool.tile([128, CW], F32, tag=f"m{c}")
                nc.vector.scalar_tensor_tensor(
                    m_t[:], Ps[c][:, 2 * CW:3 * CW], m_scalar, rzs[c][:, 0:CW],
                    ALU.add, ALU.mult
                )
                ms.append(m_t)
            if enc:
                for c in range(NCH):
                    s_t = gru_pool.tile([128, CW], F32, tag=f"s{c}")
                    nc.vector.tensor_tensor(s_t[:], ms[c][:],
                                            Ps[c][:, 3 * CW:4 * CW], ALU.add)
                    ss.append(s_t)
            else:
                ss = ms
            for c in range(NCH):
                n_t = gru_pool.tile([128, CW], BF16, tag=f"n{c}")
                nc.scalar.activation(n_t[:], ss[c][:], AF.Tanh, bias=tanh_bias)
                ns_.append(n_t)
            for c in range(NCH):
                zh = gru_pool.tile([128, CW], BF16, tag=f"zh{c}")
                nc.vector.tensor_tensor(zh[:], rzs[c][:, CW:2 * CW], hs[c][:],
                                        ALU.mult)
                zhs.append(zh)
            for c in range(NCH):
                zn = gru_pool.tile([128, CW], BF16, tag=f"zn{c}")
                nc.vector.scalar_tensor_tensor(
                    zn[:], rzs[c][:, CW:2 * CW], 1.0, ns_[c][:],
                    ALU.subtract, ALU.mult
                )
                zns.append(zn)
            for c in range(NCH):
                h_new = gru_pool.tile([128, CW], BF16, tag=f"h{c}")
                nc.vector.tensor_tensor(h_new[:], zhs[c][:], zns[c][:],
                                        ALU.subtract)
                hnew.append(h_new)
            return hnew

        # schedule: 34 GRU steps; GCN tile j rides step floor(j*25/16)
        gcn_at = {}
        for j in range(NTILE):
            gcn_at.setdefault((j * 25) // NTILE, []).append(j)

        hs = []
        for c in range(NCH):
            h0 = gru_pool.tile([128, CW], BF16, tag=f"h{c}", name=f"h0_{c}")
            nc.vector.memset(h0[:], 0.0)
            hs.append(h0)
        enc_pre = encpre_t[:] if enc_preload else None

        mx = pt("mx", [BS, 1])
        negmx = pt("negmx", [BS, 1])
        ex = pt("ex", [BS, N])
        ssum = pt("ssum", [BS, 1])
        rsum = pt("rsum", [BS, 1])
        hd = hd_ps.tile([T, BS], F32, tag="hd")

        soft_emitted = 0
        for step in range(T + 2 + T):
            if step < T + 2:
                hs = gru_step_both(hs, step * BS, whhTe_t, wihTe_t, enc_pre,
                                   binE_t[:, 0:1], bhnE_t[:, 0:1], enc=True)
            else:
                t = step - (T + 2)
                hs = gru_step_both(hs, 0, whhTd_t, None,
                                   decpre_t[:, t * E:(t + 1) * E],
                                   btD_t[:, t:t + 1], mscD_t[:, 0:1], enc=False)
                for c in range(NCH):
                    nc.tensor.matmul(
                        hd[:, c * CW:(c + 1) * CW],
                        onehot_t[:, t * 16:(t + 1) * 16], hs[c][:],
                        start=(t == 0 and c == 0),
                        stop=(t == T - 1 and c == NCH - 1),
                        skip_group_check=True,
                    )
            for j in gcn_at.get(step, []):
                gcn_tile(j)
            if step == 24:
                # all proj matmuls done (last tile at step 23): drain + softmax
                nc.vector.tensor_copy(prsb_t[:], prp[0:32, :])
                nc.sync.dma_start(out=nsproj_t[:], in_=prsb_t[:])
                nc.vector.reduce_max(mx[:], nsproj_t[:], mybir.AxisListType.X)
                nc.vector.tensor_scalar(negmx[:], mx[:], -1.0, None, ALU.mult)
                nc.scalar.activation(ex[:], nsproj_t[:], AF.Exp,
                                     bias=negmx[:, 0:1], accum_out=ssum[:, 0:1])
                nc.vector.reciprocal(rsum[:], ssum[:])
            if step >= 25:
                for _ in range(2):
                    if soft_emitted < T:
                        t2 = soft_emitted
                        nc.vector.tensor_scalar(
                            soft_all[:, t2 * N:(t2 + 1) * N], ex[:],
                            rsum[:, 0:1], None, ALU.mult
                        )
                        eng = nc.sync if t2 % 2 == 0 else nc.scalar
                        eng.dma_start(out=soft_o[:, t2 * N:(t2 + 1) * N],
                                      in_=soft_all[:, t2 * N:(t2 + 1) * N])
                        soft_emitted += 1

        # ---------------- hdot drain: [t,b] -> [b,t] ----------------
        nc.vector.tensor_scalar(hd1_t[:], hd[:], mlpb16_t[:, 0:1], None, ALU.add)
        hdp = g_ps.tile([BS, T], F32, tag="gps", name="hdp")
        nc.tensor.transpose(hdp[:], hd1_t[:], eye_t[0:T, 0:T])
        nc.vector.tensor_copy(hdot_t[:], hdp[:])

        for t in range(T):
            nc.vector.tensor_scalar(
                logits_all[:, t * N:(t + 1) * N], nsproj_t[:],
                hdot_t[:, t:t + 1], None, ALU.add
            )
            eng = nc.sync if t % 2 == 0 else nc.scalar
            eng.dma_start(out=logits_o[:, t * N:(t + 1) * N],
                          in_=logits_all[:, t * N:(t + 1) * N])

    nc.finalize()  # Bacc.compile(): wait-splitting, reg alloc, nop fusion
    return nc


def _prep_inputs(inputs):
    """Host-side preprocessing -> per-core input maps + program flags."""
    f32 = np.float32
    bf16 = ml_dtypes.bfloat16
    net_x = np.ascontiguousarray(np.asarray(inputs["net_x"], dtype=f32))
    sfc_state = np.ascontiguousarray(np.asarray(inputs["sfc_state"], dtype=f32))
    edge_index = np.asarray(inputs["edge_index"]).astype(np.int64)
    source_dest = np.asarray(inputs["source_dest"]).astype(np.int64)
    node_embed = np.asarray(inputs["node_embed"], dtype=f32)
    gcn_w = np.asarray(inputs["gcn_w"], dtype=f32)
    gcn_b = np.asarray(inputs["gcn_b"], dtype=f32)
    enc_wih = np.asarray(inputs["enc_wih"], dtype=f32)
    enc_whh = np.asarray(inputs["enc_whh"], dtype=f32)
    enc_bih = np.asarray(inputs["enc_bih"], dtype=f32)
    enc_bhh = np.asarray(inputs["enc_bhh"], dtype=f32)
    dec_wih = np.asarray(inputs["dec_wih"], dtype=f32)
    dec_whh = np.asarray(inputs["dec_whh"], dtype=f32)
    dec_bih = np.asarray(inputs["dec_bih"], dtype=f32)
    dec_bhh = np.asarray(inputs["dec_bhh"], dtype=f32)
    mlp_w = np.asarray(inputs["mlp_w"], dtype=f32)
    mlp_b = np.asarray(inputs["mlp_b"], dtype=f32)

    # normalized adjacency with self-loops, transposed: AT[s, d]
    src = np.concatenate([edge_index[0], np.arange(N, dtype=np.int64)])
    dst = np.concatenate([edge_index[1], np.arange(N, dtype=np.int64)])
    deg = np.zeros(N, dtype=f32)
    np.add.at(deg, dst, f32(1.0))
    with np.errstate(divide="ignore"):
        dinv = (1.0 / np.sqrt(deg)).astype(f32)
    norm = (dinv[src] * dinv[dst]).astype(f32)
    AT = np.zeros((N, N), dtype=f32)
    np.add.at(AT, (src, dst), norm)

    embs_full = np.ascontiguousarray(node_embed[source_dest[:, 0]].T)  # [E, B]
    embd_full = np.ascontiguousarray(node_embed[source_dest[:, 1]].T)

    brE = enc_bih[0:E] + enc_bhh[0:E]
    bzE = enc_bih[E:2 * E] + enc_bhh[E:2 * E]
    enc_preload = bool(np.any(brE) or np.any(bzE))
    encpre = np.stack([brE, bzE]).astype(bf16)  # [2, E]

    gi_dec = node_embed[:T] @ dec_wih.T + dec_bih  # [T, 384]
    brD = gi_dec[:, 0:E] + dec_bhh[0:E]            # [T, E]
    bzD = gi_dec[:, E:2 * E] + dec_bhh[E:2 * E]
    decpre = np.zeros((2, T * E), dtype=bf16)
    for t in range(T):
        decpre[0, t * E:(t + 1) * E] = brD[t].astype(bf16)
        decpre[1, t * E:(t + 1) * E] = bzD[t].astype(bf16)
    btD = np.ascontiguousarray(gi_dec[:, 2 * E:3 * E].T)   # [E, T]
    mscD = np.ascontiguousarray(dec_bhh[2 * E:3 * E, None])

    selmask = np.zeros((2, 2 * CW), dtype=bf16)
    selmask[0, 0:CW] = 1
    selmask[1, CW:2 * CW] = 1

    mlp_w_bf = mlp_w.astype(bf16)
    onehot = np.zeros((E, 16 * T), dtype=bf16)
    for t in range(T):
        onehot[:, t * 16 + t] = mlp_w_bf
    ohproj = np.zeros((E, 32 * 32), dtype=bf16)
    for g in range(32):
        ohproj[:, g * 32 + g] = mlp_w_bf

    shared = {
        "at": AT,
        "gcnw": np.vstack([gcn_w, gcn_w]).astype(bf16),
        "gcnb": np.ascontiguousarray(gcn_b[:, None]),
        "onehot": onehot,
        "ohproj": ohproj,
        "wihTe": np.ascontiguousarray(enc_wih.T).astype(bf16),
        "whhTe": np.ascontiguousarray(enc_whh.T).astype(bf16),
        "whhTd": np.ascontiguousarray(dec_whh.T).astype(bf16),
        "encpre": encpre,
        "decpre": decpre,
        "selmask": selmask,
        "binE": np.ascontiguousarray(enc_bih[2 * E:3 * E, None]),
        "bhnE": np.ascontiguousarray(enc_bhh[2 * E:3 * E, None]),
        "btD": btD,
        "mscD": mscD,
        "mlpb16": np.full((T, 1), float(mlp_b), dtype=f32),
        "eye": np.eye(128, dtype=f32),
    }
    in_maps = []
    for c in range(NCORES):
        b0 = c * BS
        m = dict(shared)
        m["netx"] = np.ascontiguousarray(net_x[b0:b0 + BS])
        m["sfc"] = np.ascontiguousarray(sfc_state[b0:b0 + BS])
        m["embs"] = np.ascontiguousarray(embs_full[:, b0:b0 + BS])
        m["embd"] = np.ascontiguousarray(embd_full[:, b0:b0 + BS])
        in_maps.append(m)
    return in_maps, enc_preload


def _get_program(enc_preload: bool):
    key = ("v2", enc_preload)
    if key not in _CACHE:
        _CACHE[key] = _build_program(enc_preload)
    return _CACHE[key]


def run(inputs, trace=False, tmpdir=None):
    in_maps, enc_preload = _prep_inputs(inputs)
    nc = _get_program(enc_preload)
    res = run_bass_kernel_spmd(
        nc, in_maps, list(range(NCORES)), trace=trace, tmpdir=tmpdir
    )
    logits = np.concatenate(
        [res.results[c]["logits"].reshape(BS, T, N) for c in range(NCORES)], axis=0
    )
    soft = np.concatenate(
        [res.results[c]["soft"].reshape(BS, T, N) for c in range(NCORES)], axis=0
    )
    return (logits, soft), res


def kernel(**inputs):
    (logits, soft), _ = run(inputs, trace=False)
    return logits, soft
